# revision 30
# baseline (speedup 1.0000x reference)
"""2-layer LSTM greedy decoder (H=4096, E=512, 15 steps) on 8 trn2 NeuronCores.

Tensor-parallel over the 4*H gate dimension: core c owns rows
{b*H + c*512 + l} of each gate block b, so the AllGather of the per-core
h-slices lands in plain h order (no permutations anywhere).

Single-pass fp16 weights as the matmul *moving* operand (1 cyc/row on the
PE), stationary operand is the fp16 h vector column (M=1) -> one PSUM row
per layer, activations read PSUM directly.  The h state travels in fp16
end-to-end (tail -> AllGather -> [P,Kh] stationary tile).

The input projection W_ih0 @ x is not computed on device at all: x is
always an embedding row, so z_x[tok] = W_ih0 @ embed[tok] + b0 is
precomputed on the host ([VOCAB, Gc] per core) and fetched by one
indirect-DMA row lookup per step, folded into the PSUM accumulation with
a contraction-1 matmul.

R_RES of the 96 weight chunks stay SBUF-resident (loaded during step 0);
the rest stream from HBM each step over both hardware DGE rings.  The
last few chunks of each step go SP-only so the ACT ring is clear for the
critical AllGather/argmax path at the step boundary.

Numerically validated in numpy: rel err ~1.7e-3, zero greedy-token flips,
worst argmax margin/noise ~6.
"""

import numpy as np

H = 4096
E = 512
T = 15
NCORES = 8
P = 128
R_RES = 40          # SBUF-resident weight chunks (of KT = 96)


def chunk_split(kt, r_res):
    """Evenly-spread resident set; returns (res_rank, str_rank) dicts
    mapping global chunk idx -> position within wres / wstr."""
    res_rank, str_rank = {}, {}
    for i in range(kt):
        if (i * r_res) // kt != ((i + 1) * r_res) // kt:
            res_rank[i] = len(res_rank)
        else:
            str_rank[i] = len(str_rank)
    return res_rank, str_rank


def build_nc(h=H, e=E, t_steps=T, ncores=NCORES, r_res=R_RES):
    import concourse.bass as bass
    import concourse.mybir as mybir
    import concourse.tile as tile
    from concourse import bacc, bass_isa

    dt = mybir.dt
    AF = mybir.ActivationFunctionType
    OP = mybir.AluOpType

    Kh = h // P                      # 32
    KT = 3 * Kh                      # 96 chunks: [L0h | L1h1 | L1h0]
    Hc = h // ncores                 # 512
    Gc = 4 * Hc                      # 2048 gate rows per core
    NSZ = 512
    NB = Gc // NSZ                   # 4 psum banks per layer
    f32, f16, u32 = dt.float32, dt.float16, dt.uint32
    SIG, TANH = AF.Sigmoid, AF.Tanh

    nc = bacc.Bacc("TRN2", target_bir_lowering=False, debug=False,
                   num_devices=ncores)

    wres_d = nc.dram_tensor("wres", [P, r_res * Gc], f16, kind="ExternalInput")
    wstr_d = nc.dram_tensor("wstr", [P, (KT - r_res) * Gc], f16,
                            kind="ExternalInput")
    b16_d = nc.dram_tensor("b16", [1, Gc], f16, kind="ExternalInput")
    zxt_d = nc.dram_tensor("zxt", [h, Gc], f16, kind="ExternalInput")
    zx0_d = nc.dram_tensor("zx0", [1, Gc], f16, kind="ExternalInput")
    ones_d = nc.dram_tensor("ones", [1, 1], f16, kind="ExternalInput")
    iota_d = nc.dram_tensor("iotas", [P, 2], f32, kind="ExternalInput")
    outd = nc.dram_tensor("out", [t_steps, Hc], f16, kind="ExternalOutput")

    BIG = 8192.0

    with tile.TileContext(nc) as tc, \
            tc.tile_pool(name="wsa", bufs=2) as wsa, \
            tc.tile_pool(name="wsb", bufs=2) as wsb, \
            tc.tile_pool(name="hx", bufs=2) as hxp, \
            tc.tile_pool(name="zx", bufs=1) as zxp, \
            tc.tile_pool(name="gat", bufs=1) as gatp, \
            tc.tile_pool(name="small", bufs=1) as smp, \
            tc.tile_pool(name="hout", bufs=1) as hop, \
            tc.tile_pool(name="amx", bufs=2) as amxp, \
            tc.tile_pool(name="const", bufs=1) as cstp, \
            tc.tile_pool(name="ps0", bufs=1, space="PSUM") as psp0, \
            tc.tile_pool(name="ps1", bufs=1, space="PSUM") as psp1, \
            tc.tile_pool(name="dram", bufs=2, space="DRAM") as drp:

        # ---- constants / persistent state ----
        b16 = cstp.tile([1, Gc], f16, tag="b16", name="b16")
        nc.scalar.dma_start(out=b16[:, :], in_=b16_d[:, :])
        ones = cstp.tile([1, 1], f16, tag="ones", name="ones")
        nc.scalar.dma_start(out=ones[:, :], in_=ones_d[:, :])
        iotas = cstp.tile([P, 2], f32, tag="iotas", name="iotas")
        nc.scalar.dma_start(out=iotas[:, :], in_=iota_d[:, :])
        c_t = {}
        for layer in (0, 1):
            c_t[layer] = cstp.tile([1, Hc], f32, tag=f"c{layer}",
                                   name=f"c{layer}")
            nc.vector.memset(c_t[layer][:, :], 0.0)

        # dummy AllGather: warms the collective machinery (plan staging is
        # ~100us on first use) and barriers the cores before the real steps
        wa_sb = smp.tile([1, 8], f16, tag="wasb", name="wasb")
        nc.vector.memset(wa_sb[:, :], 0.0)
        wa_in = drp.tile([1, 8], f16, tag="wai", name="wai")
        nc.scalar.dma_start(out=wa_in[:, :], in_=wa_sb[:, :])
        wa_out = drp.tile([1, 8 * ncores], f16, tag="wao", name="wao")
        nc.gpsimd.collective_compute(
            "AllGather", OP.bypass, replica_groups=[list(range(ncores))],
            ins=[wa_in[:, :].opt()], outs=[wa_out[:, :].opt()])

        res_rank, str_rank = chunk_split(KT, r_res)
        wres = cstp.tile([P, r_res * Gc], f16, tag="wres", name="wres")
        # L1h0 resident chunks are consumed already at t=0: load them now
        l1h0_ranks = [res_rank[i] for i in range(2 * Kh, KT) if i in res_rank]
        r0 = min(l1h0_ranks)
        nc.sync.dma_start(out=wres[:, r0 * Gc:r_res * Gc],
                          in_=wres_d[:, r0 * Gc:r_res * Gc])

        # first-step z_x from feature_vector (includes b0)
        zx16 = zxp.tile([2, Gc], f16, tag="zx", name="zx")
        nc.scalar.dma_start(out=zx16[0:1, :], in_=zx0_d[:, :])

        v16 = {0: None, 1: None}     # fp16 h vectors [P, Kh]

        def stream_plan(idxs):
            """(chunks, ring) groups: repeating [2 -> SP, 1 -> ACT], but the
            first 6 and last 8 chunks go SP-only so the ACT ring is clear
            for the critical step-boundary DMAs (agin/hT loads)."""
            lead, head, tail = idxs[:6], idxs[6:-8], idxs[-8:]
            groups = [(lead[j:j + 2], 0) for j in range(0, len(lead), 2)]
            n = 0
            i = 0
            while i < len(head):
                take = 2 if (n % 2 == 0) else 1
                take = min(take, len(head) - i)
                groups.append((head[i:i + take], n % 2))
                n += 1
                i += take
            for j in range(0, len(tail), 2):
                groups.append((tail[j:j + 2], 0))
            return groups

        def layer_mms(ps, segs, vecs):
            """segs: (chunk_base, nk, lhsT_tile); vecs: [1,Gc] f16 APs
            accumulated via contraction-1 matmuls (bias / z_x), placed
            last in the group."""
            streamed = [b + k for b, nk, _ in segs for k in range(nk)
                        if (b + k) in str_rank]
            gtiles = {}
            for g, ring in stream_plan(streamed):
                eng = nc.sync if ring == 0 else nc.scalar
                pool = wsa if ring == 0 else wsb
                wt = pool.tile([P, len(g) * Gc], f16,
                               tag=f"w{ring}", name="wst")
                eng.dma_start(
                    out=wt[:, :],
                    in_=wstr_d[:, str_rank[g[0]] * Gc:
                               (str_rank[g[-1]] + 1) * Gc])
                for j, idx in enumerate(g):
                    gtiles[idx] = wt[:, j * Gc:(j + 1) * Gc]
            first = True
            for base, nk, lt in segs:
                for k in range(nk):
                    idx = base + k
                    w = gtiles.get(idx)
                    if w is None:
                        w = wres[:, res_rank[idx] * Gc:
                                 (res_rank[idx] + 1) * Gc]
                    for n in range(NB):
                        nc.tensor.matmul(
                            ps[0:1, n * NSZ:(n + 1) * NSZ],
                            lhsT=lt[:, k:k + 1],
                            rhs=w[:, n * NSZ:(n + 1) * NSZ],
                            start=first, stop=False)
                    first = False
            for vi, vec in enumerate(vecs):
                stop = vi == len(vecs) - 1
                for n in range(NB):
                    nc.tensor.matmul(
                        ps[0:1, n * NSZ:(n + 1) * NSZ],
                        lhsT=ones[0:1, 0:1],
                        rhs=vec[0:1, n * NSZ:(n + 1) * NSZ],
                        start=first, stop=stop)
                first = False
            return ps

        def layer_tail(ps, layer):
            # gate row order is [i, f, o, g] (host-permuted) so the three
            # sigmoids fuse into one activation call
            ga = gatp.tile([1, Gc], f16, tag="ga", name="ga")
            nc.scalar.activation(out=ga[0:1, 0:3 * Hc],
                                 in_=ps[0:1, 0:3 * Hc], func=SIG)
            nc.scalar.activation(out=ga[0:1, 3 * Hc:4 * Hc],
                                 in_=ps[0:1, 3 * Hc:4 * Hc], func=TANH)
            i_g = ga[0:1, 0:Hc]
            f_g = ga[0:1, Hc:2 * Hc]
            o_g = ga[0:1, 2 * Hc:3 * Hc]
            g_g = ga[0:1, 3 * Hc:4 * Hc]
            c = c_t[layer]
            tmp = smp.tile([1, Hc], f16, tag="tmp", name="tmp")
            nc.vector.tensor_tensor(out=c[:, :], in0=c[:, :], in1=f_g,
                                    op=OP.mult)
            nc.vector.tensor_tensor(out=tmp[:, :], in0=i_g, in1=g_g,
                                    op=OP.mult)
            nc.vector.tensor_tensor(out=c[:, :], in0=c[:, :], in1=tmp[:, :],
                                    op=OP.add)
            tch = smp.tile([1, Hc], f16, tag="tch", name="tch")
            nc.scalar.activation(out=tch[:, :], in_=c[:, :], func=TANH)
            hsb = hop.tile([1, Hc], f16, tag=f"h{layer}sb", name=f"h{layer}sb")
            nc.vector.tensor_tensor(out=hsb[:, :], in0=o_g, in1=tch[:, :],
                                    op=OP.mult)
            return hsb

        def all_gather(hsb, layer):
            agin = drp.tile([1, Hc], f16, tag=f"agi{layer}",
                            name=f"agi{layer}")
            nc.scalar.dma_start(out=agin[:, :], in_=hsb[:, :])
            agout = drp.tile([1, h], f16, tag=f"ago{layer}",
                             name=f"ago{layer}")
            nc.gpsimd.collective_compute(
                "AllGather", OP.bypass,
                replica_groups=[list(range(ncores))],
                ins=[agin[:, :].opt()], outs=[agout[:, :].opt()])
            hT = hxp.tile([P, Kh], f16, tag=f"h{layer}T", name=f"h{layer}T")
            nc.scalar.dma_start(
                out=hT[:, :],
                in_=agout[:, :].rearrange("o (p k) -> (o p) k", p=P))
            v16[layer] = hT
            return hT

        for t in range(t_steps):
            if t == 1:
                # remaining resident weights load behind step 0's stream;
                # consumed from t=1 (L0h chunks come first in rank order)
                for q in range(4):
                    lo = (r0 * q) // 4
                    hi = (r0 * (q + 1)) // 4
                    nc.sync.dma_start(out=wres[:, lo * Gc:hi * Gc],
                                      in_=wres_d[:, lo * Gc:hi * Gc])

            # ---------- layer 0: gates = z_x[tok] + Whh0@h0 -------------
            ps = psp0.tile([1, Gc], f32, tag="ps0", name="ps0")
            segs = [(0, Kh, v16[0])] if t > 0 else []
            layer_mms(ps, segs, [zx16[0:1, :]])
            h0sb = layer_tail(ps, 0)
            all_gather(h0sb, 0)

            # ---------- layer 1: gates = b1 + Whh1@h1 + Wih1@h0 ----------
            ps = psp1.tile([1, Gc], f32, tag="ps1", name="ps1")
            segs = [(Kh, Kh, v16[1])] if t > 0 else []
            segs.append((2 * Kh, Kh, v16[0]))
            layer_mms(ps, segs, [b16[0:1, :]])
            h1sb = layer_tail(ps, 1)
            nc.scalar.dma_start(out=outd.ap()[t:t + 1, :], in_=h1sb[:, :])

            if t == t_steps - 1:
                break

            hT1 = all_gather(h1sb, 1)

            # ---------- argmax over full h1 + z_x row fetch --------------
            mx8 = amxp.tile([P, 8], f32, tag="mx8", name="mx8")
            mi8 = amxp.tile([P, 8], u32, tag="mi8", name="mi8")
            nc.vector.max(out=mx8[:, :], in_=hT1[:, :])
            nc.vector.max_index(out=mi8[:, :], in_max=mx8[:, :],
                                in_values=hT1[:, :])
            gmax = amxp.tile([P, 1], f32, tag="gmax", name="gmax")
            nc.gpsimd.partition_all_reduce(gmax[:, :], mx8[:, 0:1],
                                           channels=P,
                                           reduce_op=bass_isa.ReduceOp.max)
            isge = amxp.tile([P, 1], f32, tag="isge", name="isge")
            nc.vector.tensor_tensor(out=isge[:, :], in0=mx8[:, 0:1],
                                    in1=gmax[:, :], op=OP.is_ge)
            # cand = 32*p + k*  (flat h index); score = isge * (BIG - cand)
            cand = amxp.tile([P, 1], f32, tag="cand", name="cand")
            nc.vector.tensor_copy(out=cand[:, :], in_=mi8[:, 0:1])
            nc.vector.tensor_tensor(out=cand[:, :], in0=cand[:, :],
                                    in1=iotas[:, 0:1], op=OP.add)
            nc.vector.tensor_scalar(out=cand[:, :], in0=cand[:, :],
                                    scalar1=-1.0, scalar2=BIG, op0=OP.mult,
                                    op1=OP.add)
            nc.vector.tensor_tensor(out=cand[:, :], in0=cand[:, :],
                                    in1=isge[:, :], op=OP.mult)
            smax = amxp.tile([P, 1], f32, tag="smax", name="smax")
            nc.gpsimd.partition_all_reduce(smax[:, :], cand[:, :],
                                           channels=P,
                                           reduce_op=bass_isa.ReduceOp.max)
            # tok = BIG - smax; fetch z_x row (2 duplicate rows: the DGE
            # rejects single-element offset tables)
            off_f = amxp.tile([2, 1], f32, tag="offf", name="offf")
            nc.vector.tensor_scalar(out=off_f[:, :], in0=smax[0:2, 0:1],
                                    scalar1=-1.0, scalar2=BIG, op0=OP.mult,
                                    op1=OP.add)
            off = amxp.tile([2, 1], u32, tag="off", name="off")
            nc.vector.tensor_copy(out=off[:, :], in_=off_f[:, :])
            zx16 = zxp.tile([2, Gc], f16, tag="zx", name="zx")
            nc.gpsimd.indirect_dma_start(
                out=zx16[:, :], out_offset=None, in_=zxt_d[:, :],
                in_offset=bass.IndirectOffsetOnAxis(ap=off[:, :], axis=0))

    nc.compile()
    return nc


# --------------------------------------------------------------------------
# host-side data prep
# --------------------------------------------------------------------------
def prep_inputs(inputs, h=H, e=E, ncores=NCORES, r_res=R_RES):
    Kh = h // P
    Hc = h // ncores
    Gc = 4 * Hc

    fv = np.asarray(inputs["feature_vector"], np.float32)
    embed = np.asarray(inputs["embed"], np.float32)
    b0 = np.asarray(inputs["b_ih0"], np.float32) + np.asarray(
        inputs["b_hh0"], np.float32)
    b1 = np.asarray(inputs["b_ih1"], np.float32) + np.asarray(
        inputs["b_hh1"], np.float32)
    W_ih0 = np.asarray(inputs["W_ih0"], np.float32)

    def tiles(Wc, K):
        # Wc [Gc, K*P] -> [P, K*Gc] fp16, chunk k column p = Wc[:, K*p + k]
        Gc_, KP = Wc.shape
        W3 = Wc.reshape(Gc_, P, K)          # [g, p, k]
        return np.ascontiguousarray(
            W3.transpose(1, 2, 0).reshape(P, K * Gc_).astype(np.float16))

    iotas = np.stack([32.0 * np.arange(P), 1.0 * np.arange(P)],
                     axis=1).astype(np.float32)
    shared = {"ones": np.ones((1, 1), np.float16), "iotas": iotas}

    in_maps = []
    for c in range(ncores):
        rows = np.concatenate(
            [b * h + c * Hc + np.arange(Hc) for b in (0, 1, 3, 2)])
        w0h = tiles(np.asarray(inputs["W_hh0"], np.float32)[rows], Kh)
        w1h = tiles(np.asarray(inputs["W_hh1"], np.float32)[rows], Kh)
        w1x = tiles(np.asarray(inputs["W_ih1"], np.float32)[rows], Kh)
        wfull = np.concatenate([w0h, w1h, w1x], axis=1)
        kt = wfull.shape[1] // Gc
        res_rank, str_rank = chunk_split(kt, r_res)
        wcols = wfull.reshape(P, kt, Gc)
        wres = wcols[:, sorted(res_rank, key=res_rank.get), :]
        wstr = wcols[:, sorted(str_rank, key=str_rank.get), :]
        Wi = W_ih0[rows]                          # [Gc, E]
        zxt = (embed @ Wi.T + b0[rows]).astype(np.float16)   # [VOCAB, Gc]
        zx0 = (Wi @ fv + b0[rows]).reshape(1, -1).astype(np.float16)
        in_maps.append(dict(
            shared,
            wres=np.ascontiguousarray(wres.reshape(P, -1)),
            wstr=np.ascontiguousarray(wstr.reshape(P, -1)),
            b16=b1[rows].reshape(1, -1).astype(np.float16),
            zxt=np.ascontiguousarray(zxt),
            zx0=zx0))
    return in_maps


_NC_CACHE = {}


def _get_nc():
    if "nc" not in _NC_CACHE:
        _NC_CACHE["nc"] = build_nc()
    return _NC_CACHE["nc"]


def run(inputs, trace=False):
    from concourse.bass_utils import run_bass_kernel_spmd
    nc = _get_nc()
    in_maps = prep_inputs(inputs)
    res = run_bass_kernel_spmd(nc, in_maps, core_ids=list(range(NCORES)),
                               trace=trace)
    full = np.concatenate([res.results[c]["out"] for c in range(NCORES)],
                          axis=1)
    return np.ascontiguousarray(full.astype(np.float32)), res


def kernel(**inputs):
    full, _ = run(inputs, trace=False)
    return full


# revision 31
# speedup vs baseline: 1.0345x; 1.0345x over previous
"""2-layer LSTM greedy decoder (H=4096, E=512, 15 steps) on 8 trn2 NeuronCores.

Tensor-parallel over the 4*H gate dimension: core c owns rows
{b*H + c*512 + l} of each gate block b, so the AllGather of the per-core
h-slices lands in plain h order (no permutations anywhere).

Single-pass fp16 weights as the matmul *moving* operand (1 cyc/row on the
PE), stationary operand is the fp16 h vector column (M=1) -> one PSUM row
per layer, activations read PSUM directly.  The h state travels in fp16
end-to-end (tail -> AllGather -> [P,Kh] stationary tile).

The input projection W_ih0 @ x is not computed on device at all: x is
always an embedding row, so z_x[tok] = W_ih0 @ embed[tok] + b0 is
precomputed on the host ([VOCAB, Gc] per core) and fetched by one
indirect-DMA row lookup per step, folded into the PSUM accumulation with
a contraction-1 matmul.

R_RES of the 96 weight chunks stay SBUF-resident (loaded during step 0);
the rest stream from HBM each step over both hardware DGE rings.  The
last few chunks of each step go SP-only so the ACT ring is clear for the
critical AllGather/argmax path at the step boundary.

Numerically validated in numpy: rel err ~1.7e-3, zero greedy-token flips,
worst argmax margin/noise ~6.
"""

import numpy as np

H = 4096
E = 512
T = 15
NCORES = 8
P = 128
R_RES = 40          # SBUF-resident weight chunks (of KT = 96)


def chunk_split(kt, r_res):
    """Evenly-spread resident set; returns (res_rank, str_rank) dicts
    mapping global chunk idx -> position within wres / wstr."""
    res_rank, str_rank = {}, {}
    for i in range(kt):
        if (i * r_res) // kt != ((i + 1) * r_res) // kt:
            res_rank[i] = len(res_rank)
        else:
            str_rank[i] = len(str_rank)
    return res_rank, str_rank


def build_nc(h=H, e=E, t_steps=T, ncores=NCORES, r_res=R_RES):
    import concourse.bass as bass
    import concourse.mybir as mybir
    import concourse.tile as tile
    from concourse import bacc, bass_isa

    dt = mybir.dt
    AF = mybir.ActivationFunctionType
    OP = mybir.AluOpType

    Kh = h // P                      # 32
    KT = 3 * Kh                      # 96 chunks: [L0h | L1h1 | L1h0]
    Hc = h // ncores                 # 512
    Gc = 4 * Hc                      # 2048 gate rows per core
    NSZ = 512
    NB = Gc // NSZ                   # 4 psum banks per layer
    f32, f16, u32 = dt.float32, dt.float16, dt.uint32
    SIG, TANH = AF.Sigmoid, AF.Tanh

    nc = bacc.Bacc("TRN2", target_bir_lowering=False, debug=False,
                   num_devices=ncores)

    wres_d = nc.dram_tensor("wres", [P, r_res * Gc], f16, kind="ExternalInput")
    wstr_d = nc.dram_tensor("wstr", [P, (KT - r_res) * Gc], f16,
                            kind="ExternalInput")
    b16_d = nc.dram_tensor("b16", [1, Gc], f16, kind="ExternalInput")
    zxt_d = nc.dram_tensor("zxt", [h, Gc], f16, kind="ExternalInput")
    zx0_d = nc.dram_tensor("zx0", [1, Gc], f16, kind="ExternalInput")
    ones_d = nc.dram_tensor("ones", [1, 1], f16, kind="ExternalInput")
    iota_d = nc.dram_tensor("iotas", [P, 2], f32, kind="ExternalInput")
    outd = nc.dram_tensor("out", [t_steps, Hc], f16, kind="ExternalOutput")

    BIG = 8192.0

    with tile.TileContext(nc) as tc, \
            tc.tile_pool(name="wsa", bufs=2) as wsa, \
            tc.tile_pool(name="wsb", bufs=2) as wsb, \
            tc.tile_pool(name="hx", bufs=2) as hxp, \
            tc.tile_pool(name="zx", bufs=1) as zxp, \
            tc.tile_pool(name="gat", bufs=1) as gatp, \
            tc.tile_pool(name="small", bufs=1) as smp, \
            tc.tile_pool(name="hout", bufs=1) as hop, \
            tc.tile_pool(name="amx", bufs=2) as amxp, \
            tc.tile_pool(name="const", bufs=1) as cstp, \
            tc.tile_pool(name="ps0", bufs=1, space="PSUM") as psp0, \
            tc.tile_pool(name="ps1", bufs=1, space="PSUM") as psp1, \
            tc.tile_pool(name="dram", bufs=2, space="DRAM") as drp:

        # ---- constants / persistent state ----
        b16 = cstp.tile([1, Gc], f16, tag="b16", name="b16")
        nc.scalar.dma_start(out=b16[:, :], in_=b16_d[:, :])
        ones = cstp.tile([1, 1], f16, tag="ones", name="ones")
        nc.scalar.dma_start(out=ones[:, :], in_=ones_d[:, :])
        iotas = cstp.tile([P, 2], f32, tag="iotas", name="iotas")
        nc.scalar.dma_start(out=iotas[:, :], in_=iota_d[:, :])
        c_t = {}
        for layer in (0, 1):
            c_t[layer] = cstp.tile([1, Hc], f32, tag=f"c{layer}",
                                   name=f"c{layer}")
            nc.vector.memset(c_t[layer][:, :], 0.0)

        # dummy AllGather: warms the collective machinery (plan staging is
        # ~100us on first use) and barriers the cores before the real steps
        wa_sb = smp.tile([1, 8], f16, tag="wasb", name="wasb")
        nc.vector.memset(wa_sb[:, :], 0.0)
        wa_in = drp.tile([1, 8], f16, tag="wai", name="wai")
        nc.scalar.dma_start(out=wa_in[:, :], in_=wa_sb[:, :])
        wa_out = drp.tile([1, 8 * ncores], f16, tag="wao", name="wao")
        nc.gpsimd.collective_compute(
            "AllGather", OP.bypass, replica_groups=[list(range(ncores))],
            ins=[wa_in[:, :].opt()], outs=[wa_out[:, :].opt()])

        res_rank, str_rank = chunk_split(KT, r_res)
        wres = cstp.tile([P, r_res * Gc], f16, tag="wres", name="wres")
        # L1h0 resident chunks are consumed already at t=0: load them now
        l1h0_ranks = [res_rank[i] for i in range(2 * Kh, KT) if i in res_rank]
        r0 = min(l1h0_ranks)
        nc.sync.dma_start(out=wres[:, r0 * Gc:r_res * Gc],
                          in_=wres_d[:, r0 * Gc:r_res * Gc])

        # first-step z_x from feature_vector (includes b0)
        zx16 = zxp.tile([2, Gc], f16, tag="zx", name="zx")
        nc.scalar.dma_start(out=zx16[0:1, :], in_=zx0_d[:, :])

        v16 = {0: None, 1: None}     # fp16 h vectors [P, Kh]

        def stream_plan(idxs):
            """(chunks, ring) groups: repeating [2 -> SP, 1 -> ACT], but the
            first 4 and last 6 chunks go SP-only so the ACT ring is clear
            for the critical step-boundary DMAs (agin/hT loads)."""
            lead, head, tail = idxs[:4], idxs[4:-6], idxs[-6:]
            groups = [(lead[j:j + 2], 0) for j in range(0, len(lead), 2)]
            n = 0
            i = 0
            while i < len(head):
                take = 2 if (n % 2 == 0) else 1
                take = min(take, len(head) - i)
                groups.append((head[i:i + take], n % 2))
                n += 1
                i += take
            for j in range(0, len(tail), 2):
                groups.append((tail[j:j + 2], 0))
            return groups

        def layer_mms(ps, segs, vecs):
            """segs: (chunk_base, nk, lhsT_tile); vecs: [1,Gc] f16 APs
            accumulated via contraction-1 matmuls (bias / z_x), placed
            last in the group."""
            streamed = [b + k for b, nk, _ in segs for k in range(nk)
                        if (b + k) in str_rank]
            gtiles = {}
            for g, ring in stream_plan(streamed):
                eng = nc.sync if ring == 0 else nc.scalar
                pool = wsa if ring == 0 else wsb
                wt = pool.tile([P, len(g) * Gc], f16,
                               tag=f"w{ring}", name="wst")
                eng.dma_start(
                    out=wt[:, :],
                    in_=wstr_d[:, str_rank[g[0]] * Gc:
                               (str_rank[g[-1]] + 1) * Gc])
                for j, idx in enumerate(g):
                    gtiles[idx] = wt[:, j * Gc:(j + 1) * Gc]
            first = True
            for base, nk, lt in segs:
                for k in range(nk):
                    idx = base + k
                    w = gtiles.get(idx)
                    if w is None:
                        w = wres[:, res_rank[idx] * Gc:
                                 (res_rank[idx] + 1) * Gc]
                    for n in range(NB):
                        nc.tensor.matmul(
                            ps[0:1, n * NSZ:(n + 1) * NSZ],
                            lhsT=lt[:, k:k + 1],
                            rhs=w[:, n * NSZ:(n + 1) * NSZ],
                            start=first, stop=False)
                    first = False
            for vi, vec in enumerate(vecs):
                stop = vi == len(vecs) - 1
                for n in range(NB):
                    nc.tensor.matmul(
                        ps[0:1, n * NSZ:(n + 1) * NSZ],
                        lhsT=ones[0:1, 0:1],
                        rhs=vec[0:1, n * NSZ:(n + 1) * NSZ],
                        start=first, stop=stop)
                first = False
            return ps

        def layer_tail(ps, layer):
            # gate row order is [i, f, o, g] (host-permuted) so the three
            # sigmoids fuse into one activation call
            ga = gatp.tile([1, Gc], f16, tag="ga", name="ga")
            nc.scalar.activation(out=ga[0:1, 0:3 * Hc],
                                 in_=ps[0:1, 0:3 * Hc], func=SIG)
            nc.scalar.activation(out=ga[0:1, 3 * Hc:4 * Hc],
                                 in_=ps[0:1, 3 * Hc:4 * Hc], func=TANH)
            i_g = ga[0:1, 0:Hc]
            f_g = ga[0:1, Hc:2 * Hc]
            o_g = ga[0:1, 2 * Hc:3 * Hc]
            g_g = ga[0:1, 3 * Hc:4 * Hc]
            c = c_t[layer]
            tmp = smp.tile([1, Hc], f16, tag="tmp", name="tmp")
            nc.vector.tensor_tensor(out=c[:, :], in0=c[:, :], in1=f_g,
                                    op=OP.mult)
            nc.vector.tensor_tensor(out=tmp[:, :], in0=i_g, in1=g_g,
                                    op=OP.mult)
            nc.vector.tensor_tensor(out=c[:, :], in0=c[:, :], in1=tmp[:, :],
                                    op=OP.add)
            tch = smp.tile([1, Hc], f16, tag="tch", name="tch")
            nc.scalar.activation(out=tch[:, :], in_=c[:, :], func=TANH)
            hsb = hop.tile([1, Hc], f16, tag=f"h{layer}sb", name=f"h{layer}sb")
            nc.vector.tensor_tensor(out=hsb[:, :], in0=o_g, in1=tch[:, :],
                                    op=OP.mult)
            return hsb

        def all_gather(hsb, layer):
            agin = drp.tile([1, Hc], f16, tag=f"agi{layer}",
                            name=f"agi{layer}")
            nc.scalar.dma_start(out=agin[:, :], in_=hsb[:, :])
            agout = drp.tile([1, h], f16, tag=f"ago{layer}",
                             name=f"ago{layer}")
            nc.gpsimd.collective_compute(
                "AllGather", OP.bypass,
                replica_groups=[list(range(ncores))],
                ins=[agin[:, :].opt()], outs=[agout[:, :].opt()])
            hT = hxp.tile([P, Kh], f16, tag=f"h{layer}T", name=f"h{layer}T")
            nc.scalar.dma_start(
                out=hT[:, :],
                in_=agout[:, :].rearrange("o (p k) -> (o p) k", p=P))
            v16[layer] = hT
            return hT

        for t in range(t_steps):
            if t == 1:
                # remaining resident weights load behind step 0's stream;
                # consumed from t=1 (L0h chunks come first in rank order)
                for q in range(4):
                    lo = (r0 * q) // 4
                    hi = (r0 * (q + 1)) // 4
                    nc.sync.dma_start(out=wres[:, lo * Gc:hi * Gc],
                                      in_=wres_d[:, lo * Gc:hi * Gc])

            # ---------- layer 0: gates = z_x[tok] + Whh0@h0 -------------
            ps = psp0.tile([1, Gc], f32, tag="ps0", name="ps0")
            segs = [(0, Kh, v16[0])] if t > 0 else []
            layer_mms(ps, segs, [zx16[0:1, :]])
            h0sb = layer_tail(ps, 0)
            all_gather(h0sb, 0)

            # ---------- layer 1: gates = b1 + Whh1@h1 + Wih1@h0 ----------
            ps = psp1.tile([1, Gc], f32, tag="ps1", name="ps1")
            segs = [(Kh, Kh, v16[1])] if t > 0 else []
            segs.append((2 * Kh, Kh, v16[0]))
            layer_mms(ps, segs, [b16[0:1, :]])
            h1sb = layer_tail(ps, 1)
            nc.scalar.dma_start(out=outd.ap()[t:t + 1, :], in_=h1sb[:, :])

            if t == t_steps - 1:
                break

            hT1 = all_gather(h1sb, 1)

            # ---------- argmax over full h1 + z_x row fetch --------------
            mx8 = amxp.tile([P, 8], f32, tag="mx8", name="mx8")
            mi8 = amxp.tile([P, 8], u32, tag="mi8", name="mi8")
            nc.vector.max(out=mx8[:, :], in_=hT1[:, :])
            nc.vector.max_index(out=mi8[:, :], in_max=mx8[:, :],
                                in_values=hT1[:, :])
            gmax = amxp.tile([P, 1], f32, tag="gmax", name="gmax")
            nc.gpsimd.partition_all_reduce(gmax[:, :], mx8[:, 0:1],
                                           channels=P,
                                           reduce_op=bass_isa.ReduceOp.max)
            isge = amxp.tile([P, 1], f32, tag="isge", name="isge")
            nc.vector.tensor_tensor(out=isge[:, :], in0=mx8[:, 0:1],
                                    in1=gmax[:, :], op=OP.is_ge)
            # cand = 32*p + k*  (flat h index); score = isge * (BIG - cand)
            cand = amxp.tile([P, 1], f32, tag="cand", name="cand")
            nc.vector.tensor_copy(out=cand[:, :], in_=mi8[:, 0:1])
            nc.vector.tensor_tensor(out=cand[:, :], in0=cand[:, :],
                                    in1=iotas[:, 0:1], op=OP.add)
            nc.vector.tensor_scalar(out=cand[:, :], in0=cand[:, :],
                                    scalar1=-1.0, scalar2=BIG, op0=OP.mult,
                                    op1=OP.add)
            nc.vector.tensor_tensor(out=cand[:, :], in0=cand[:, :],
                                    in1=isge[:, :], op=OP.mult)
            smax = amxp.tile([P, 1], f32, tag="smax", name="smax")
            nc.gpsimd.partition_all_reduce(smax[:, :], cand[:, :],
                                           channels=P,
                                           reduce_op=bass_isa.ReduceOp.max)
            # tok = BIG - smax; fetch z_x row (2 duplicate rows: the DGE
            # rejects single-element offset tables)
            off_f = amxp.tile([2, 1], f32, tag="offf", name="offf")
            nc.vector.tensor_scalar(out=off_f[:, :], in0=smax[0:2, 0:1],
                                    scalar1=-1.0, scalar2=BIG, op0=OP.mult,
                                    op1=OP.add)
            off = amxp.tile([2, 1], u32, tag="off", name="off")
            nc.vector.tensor_copy(out=off[:, :], in_=off_f[:, :])
            zx16 = zxp.tile([2, Gc], f16, tag="zx", name="zx")
            nc.gpsimd.indirect_dma_start(
                out=zx16[:, :], out_offset=None, in_=zxt_d[:, :],
                in_offset=bass.IndirectOffsetOnAxis(ap=off[:, :], axis=0))

    nc.compile()
    return nc


# --------------------------------------------------------------------------
# host-side data prep
# --------------------------------------------------------------------------
def prep_inputs(inputs, h=H, e=E, ncores=NCORES, r_res=R_RES):
    Kh = h // P
    Hc = h // ncores
    Gc = 4 * Hc

    fv = np.asarray(inputs["feature_vector"], np.float32)
    embed = np.asarray(inputs["embed"], np.float32)
    b0 = np.asarray(inputs["b_ih0"], np.float32) + np.asarray(
        inputs["b_hh0"], np.float32)
    b1 = np.asarray(inputs["b_ih1"], np.float32) + np.asarray(
        inputs["b_hh1"], np.float32)
    W_ih0 = np.asarray(inputs["W_ih0"], np.float32)

    def tiles(Wc, K):
        # Wc [Gc, K*P] -> [P, K*Gc] fp16, chunk k column p = Wc[:, K*p + k]
        Gc_, KP = Wc.shape
        W3 = Wc.reshape(Gc_, P, K)          # [g, p, k]
        return np.ascontiguousarray(
            W3.transpose(1, 2, 0).reshape(P, K * Gc_).astype(np.float16))

    iotas = np.stack([32.0 * np.arange(P), 1.0 * np.arange(P)],
                     axis=1).astype(np.float32)
    shared = {"ones": np.ones((1, 1), np.float16), "iotas": iotas}

    in_maps = []
    for c in range(ncores):
        rows = np.concatenate(
            [b * h + c * Hc + np.arange(Hc) for b in (0, 1, 3, 2)])
        w0h = tiles(np.asarray(inputs["W_hh0"], np.float32)[rows], Kh)
        w1h = tiles(np.asarray(inputs["W_hh1"], np.float32)[rows], Kh)
        w1x = tiles(np.asarray(inputs["W_ih1"], np.float32)[rows], Kh)
        wfull = np.concatenate([w0h, w1h, w1x], axis=1)
        kt = wfull.shape[1] // Gc
        res_rank, str_rank = chunk_split(kt, r_res)
        wcols = wfull.reshape(P, kt, Gc)
        wres = wcols[:, sorted(res_rank, key=res_rank.get), :]
        wstr = wcols[:, sorted(str_rank, key=str_rank.get), :]
        Wi = W_ih0[rows]                          # [Gc, E]
        zxt = (embed @ Wi.T + b0[rows]).astype(np.float16)   # [VOCAB, Gc]
        zx0 = (Wi @ fv + b0[rows]).reshape(1, -1).astype(np.float16)
        in_maps.append(dict(
            shared,
            wres=np.ascontiguousarray(wres.reshape(P, -1)),
            wstr=np.ascontiguousarray(wstr.reshape(P, -1)),
            b16=b1[rows].reshape(1, -1).astype(np.float16),
            zxt=np.ascontiguousarray(zxt),
            zx0=zx0))
    return in_maps


_NC_CACHE = {}


def _get_nc():
    if "nc" not in _NC_CACHE:
        _NC_CACHE["nc"] = build_nc()
    return _NC_CACHE["nc"]


def run(inputs, trace=False):
    from concourse.bass_utils import run_bass_kernel_spmd
    nc = _get_nc()
    in_maps = prep_inputs(inputs)
    res = run_bass_kernel_spmd(nc, in_maps, core_ids=list(range(NCORES)),
                               trace=trace)
    full = np.concatenate([res.results[c]["out"] for c in range(NCORES)],
                          axis=1)
    return np.ascontiguousarray(full.astype(np.float32)), res


def kernel(**inputs):
    full, _ = run(inputs, trace=False)
    return full
